# revision 9
# baseline (speedup 1.0000x reference)
"""Trainium2 Bass kernel for CrossAttentionFusion.

Reference computation (per batch element b, torch Linear convention):
    V = Xkv @ Wv.T + bv            [Skv, D]
    K = Xkv @ Wk.T + bk            [Skv, D]
    Q = Xq  @ Wq.T + bq            [Sq, D]
    E = Q @ K.T / sqrt(128)        [Sq, Skv]
    A = softmax(E, axis=-1)
    F = A @ V                      [Sq, D]
    O = F @ Wd.T + bd              [Sq, D]

Sharding: data-parallel over batch, B=32 across 8 cores (4 per core).

This environment executes NEFFs with a large per-instruction overhead
(~50-120us/instr, engines overlapping), so the design minimizes
instruction count per engine rather than modeled cycles:

  Algebraic folds (exact):
    E = Q K^T = (Xq Wqk + 1 c^T) Xkv^T + r 1^T,  Wqk = Wq^T Wk, c = Wk^T bq
      (the r 1^T term is constant per-row -> cancels in softmax, dropped)
    O^T = M^T H^T / S + btil 1^T,  H = A_unnorm Xkv,  M = (Wd Wv)^T,
          btil = Wd bv + bd
  so the K and V projections disappear and the O projection is ONE
  stationary-M matmul per q-chunk in the transposed domain.
  Wqk/M/c/btil are precomputed on host (128x128, negligible).

  Inputs are cast to bf16 on host (error ~0.1%, budget 2e-2):
    - X^T obtained with ONE XBAR dma-transpose per tensor (no PE transposes)
    - O^T transposed back by ONE XBAR SBUF->SBUF dma-transpose per batch,
      then written out by a Pool (SWDGE) casting DMA bf16->f32
    - all big matmuls run bf16 at 1 cycle/row

  Softmax denominator: A-chunk tiles live contiguously [128,16,512];
  ONE strided DVE reduce sums the 16 kv tiles, Pool partition_all_reduce
  sums the 128 partitions (broadcasting the result), DVE reciprocal, and
  the normalization is folded into the single DVE instruction that moves
  H^T out of PSUM. exp skips max-subtraction (E ~ N(0,1), safe in fp32).

  Per batch (Sq=Skv=2048):
    PE: 4 PT + 64 E + 64 AV + 4 Oproj            = 136
    DVE: 4 PT-bias + 4 Sred + 4 recip + 4 Hmul + 4 Obias = 20
    ACT: 32 exp (1024-wide pairs)
    Pool: 4 all_reduce + 1 cast-DMA;  DMA: 4

  The PE tail (Oproj) of chunk c is deferred into chunk c+1's E-loop so
  the cross-engine S-chain latency never blocks the PE stream.
"""

import os
import numpy as np

B_TOTAL = 32
N_CORES = 8
B_PER_CORE = B_TOTAL // N_CORES
SQ = 2048
SKV = 2048
D = 128
P = 128
QCHUNK = 512
LA = 2  # AV trails E by LA kv-tiles
SCALE = 1.0 / np.sqrt(128.0)

# A/B switches (env):
#   BASS_S_MODE   = allred | reduce   (Pool all_reduce vs PE colsum chain)
#   BASS_OUT_MODE = xbar | direct     (O^T + xbar-out vs q-major oproj)
S_MODE = os.environ.get("BASS_S_MODE", "allred")
OUT_MODE = os.environ.get("BASS_OUT_MODE", "xbar")

_PROGRAM_CACHE = {}


def build_program(n_batch=B_PER_CORE, sq=SQ, skv=SKV, n_iters=1):
    import concourse.mybir as mybir
    import concourse.tile as tile
    from concourse import bacc, bass_isa
    from contextlib import ExitStack

    f32 = mybir.dt.float32
    bf16 = mybir.dt.bfloat16

    NT_Q = sq // P        # 16 q tiles per batch
    NT_KV = skv // P      # 16 kv tiles per batch
    NC_Q = sq // QCHUNK   # 4 q chunks per batch
    QSUB = QCHUNK // P    # 4 q subtiles per chunk

    nc = bacc.Bacc("TRN2", target_bir_lowering=False, debug=False)

    xq_d = nc.dram_tensor("xq", [n_batch, sq, D], bf16, kind="ExternalInput")
    xkv_d = nc.dram_tensor("xkv", [n_batch, skv, D], bf16, kind="ExternalInput")
    wqk_d = nc.dram_tensor("wqk", [D, D], bf16, kind="ExternalInput")
    m_d = nc.dram_tensor("mdv", [D, D], bf16, kind="ExternalInput")
    ccol_d = nc.dram_tensor("ccol", [D, 1], f32, kind="ExternalInput")
    btcol_d = nc.dram_tensor("btcol", [D, 1], f32, kind="ExternalInput")
    bb4_d = nc.dram_tensor("bb4", [1, QSUB * D], f32, kind="ExternalInput")
    ones_d = nc.dram_tensor("onescol", [D, 1], f32, kind="ExternalInput")
    out_d = nc.dram_tensor("out", [n_batch, sq, D], f32, kind="ExternalOutput")

    with tile.TileContext(nc) as tc, ExitStack() as ctx:
        const = ctx.enter_context(tc.tile_pool(name="const", bufs=1))
        xt_pool = ctx.enter_context(tc.tile_pool(name="xt", bufs=2))
        xin_pool = ctx.enter_context(tc.tile_pool(name="xin", bufs=2))
        pt_pool = ctx.enter_context(tc.tile_pool(name="pt", bufs=2))
        a_pool = ctx.enter_context(tc.tile_pool(name="a", bufs=2))
        sp_pool = ctx.enter_context(tc.tile_pool(name="sp", bufs=2))
        rb_pool = ctx.enter_context(tc.tile_pool(name="rb", bufs=2))
        ht_pool = ctx.enter_context(tc.tile_pool(name="ht", bufs=2))
        ot_pool = ctx.enter_context(tc.tile_pool(name="ot", bufs=2))
        os_pool = ctx.enter_context(tc.tile_pool(name="os", bufs=2))
        e_psum = ctx.enter_context(tc.tile_pool(name="e_psum", bufs=2, space="PSUM"))
        h_psum = ctx.enter_context(tc.tile_pool(name="h_psum", bufs=2, space="PSUM"))
        op_psum = ctx.enter_context(tc.tile_pool(name="op_psum", bufs=2, space="PSUM"))

        # ---- constants (host-precomputed, just DMA'd in) ----
        wqk_sb = const.tile([D, D], bf16, tag="wqk")
        nc.sync.dma_start(wqk_sb[:], wqk_d[:, :])
        m_sb = const.tile([D, D], bf16, tag="mdv")
        nc.sync.dma_start(m_sb[:], m_d[:, :])
        ccol = const.tile([D, 1], f32, tag="ccol")
        nc.sync.dma_start(ccol[:], ccol_d[:, :])
        btcol = const.tile([D, 1], f32, tag="btcol")
        nc.sync.dma_start(btcol[:], btcol_d[:, :])
        if OUT_MODE == "direct":
            bb4_row = const.tile([1, QSUB * D], f32, tag="bb4r")
            nc.sync.dma_start(bb4_row[:], bb4_d[:, :])
            bbc4 = const.tile([P, QSUB * D], f32, tag="bb4")
            nc.gpsimd.partition_broadcast(bbc4[:], bb4_row[:], channels=P)
        if S_MODE == "reduce":
            ones_col = const.tile([D, 1], f32, tag="ones")
            nc.sync.dma_start(ones_col[:], ones_d[:, :])
            s_psum = ctx.enter_context(
                tc.tile_pool(name="s_psum", bufs=1, space="PSUM"))
            r_pool = ctx.enter_context(tc.tile_pool(name="r", bufs=2))

        # deferred PE tail (O-projection) of the previous chunk
        pending = []

        def emit_schain(st):
            """S-chain + H normalization; no PE instructions (allred mode)."""
            a_chunk, h_ps = st["a_chunk"], st["h_ps"]
            SP = sp_pool.tile([P, QCHUNK], f32, tag="sp")
            nc.vector.tensor_reduce(
                SP[:], a_chunk[:].rearrange("p t q -> p q t"),
                mybir.AxisListType.X, mybir.AluOpType.add)
            rb = rb_pool.tile([P, QCHUNK], f32, tag="rb")
            if S_MODE == "allred":
                sb = sp_pool.tile([P, QCHUNK], f32, tag="sb")
                nc.gpsimd.partition_all_reduce(
                    sb[:], SP[:], channels=P, reduce_op=bass_isa.ReduceOp.add)
                nc.vector.reciprocal(rb[:], sb[:])
            else:
                s_ps = s_psum.tile([1, QCHUNK], f32, tag="s")
                nc.tensor.matmul(s_ps[0:1, :], lhsT=ones_col[:], rhs=SP[:],
                                 start=True, stop=True)
                recip = r_pool.tile([1, QCHUNK], f32, tag="r")
                nc.vector.reciprocal(recip[:], s_ps[0:1, :])
                nc.gpsimd.partition_broadcast(rb[:], recip[:], channels=P)
            ht = ht_pool.tile([P, QCHUNK], bf16, tag="ht")
            nc.vector.tensor_mul(ht[:], h_ps[:], rb[:])
            st["ht"] = ht

        def emit_oproj(st):
            b, c, ht = st["b"], st["c"], st["ht"]
            if OUT_MODE == "xbar":
                # O^T chunk = M^T H^T: ONE stationary-M matmul
                op_ps = op_psum.tile([P, QCHUNK], f32, tag="op")
                nc.tensor.matmul(op_ps[:], lhsT=m_sb[:], rhs=ht[:],
                                 start=True, stop=True)
                nc.vector.tensor_scalar_add(
                    st["oT"][:, c * QCHUNK:(c + 1) * QCHUNK], op_ps[:],
                    btcol[:])
            else:
                op_ps = op_psum.tile([P, QSUB, D], f32, tag="op")
                for j in range(QSUB):
                    nc.tensor.matmul(op_ps[:, j, :],
                                     lhsT=ht[:, j * P:(j + 1) * P],
                                     rhs=m_sb[:], start=True, stop=True)
                o_sb = os_pool.tile([P, QSUB, D], f32, tag="o")
                nc.vector.tensor_add(
                    o_sb[:].rearrange("p a b -> p (a b)"),
                    op_ps[:].rearrange("p a b -> p (a b)"),
                    bbc4[:])
                o_dst = out_d[b, c * QCHUNK:(c + 1) * QCHUNK, :].rearrange(
                    "(t p) d -> p t d", p=P)
                nc.sync.dma_start(o_dst, o_sb[:])
            if OUT_MODE == "xbar" and c == NC_Q - 1:
                # all 4 chunks of batch b written: transpose + cast out
                o_seq = os_pool.tile([P, NT_Q, D], bf16, tag="oseq")
                nc.sync.dma_start_transpose(o_seq[:], st["oT"][:])
                nc.gpsimd.dma_start(
                    out_d[b].rearrange("(t p) d -> p t d", p=P), o_seq[:])

        def flush():
            while pending:
                emit_oproj(pending.pop(0))

        # ---- per batch (n_iters>1 only for wall-clock HW timing) ----
        def issue_input_dmas(b):
            """X^T via XBAR dma-transpose; Xkv seq-major via plain DMA."""
            xkvT = xt_pool.tile([P, skv], bf16, tag="xkvT")
            xqT = xt_pool.tile([P, sq], bf16, tag="xqT")
            nc.sync.dma_start_transpose(xkvT[:], xkv_d[b])
            nc.sync.dma_start_transpose(xqT[:], xq_d[b])
            xkv_sb = xin_pool.tile([P, NT_KV, D], bf16, tag="xin")
            nc.sync.dma_start(xkv_sb[:], xkv_d[b].rearrange(
                "(t p) d -> p t d", p=P))
            return xkvT, xqT, xkv_sb

        batch_seq = [bb for _ in range(n_iters) for bb in range(n_batch)]
        prefetched = issue_input_dmas(batch_seq[0])
        for bi, b in enumerate(batch_seq):
            xkvT, xqT, xkv_sb = prefetched
            if bi + 1 < len(batch_seq):
                # issue next batch's input DMAs now (bufs=2 pools); the DMA
                # queue fills them while this batch computes
                prefetched = issue_input_dmas(batch_seq[bi + 1])

            # PT = Wqk^T Xq^T + c  [D, Sq] (feature-major q projection)
            pt = pt_pool.tile([P, sq], bf16, tag="pt")
            for cq in range(sq // QCHUNK):
                ps = h_psum.tile([P, QCHUNK], f32, tag="h")
                nc.tensor.matmul(ps[:], lhsT=wqk_sb[:],
                                 rhs=xqT[:, cq * QCHUNK:(cq + 1) * QCHUNK],
                                 start=True, stop=True)
                nc.vector.tensor_scalar_add(
                    pt[:, cq * QCHUNK:(cq + 1) * QCHUNK], ps[:], ccol[:])

            oT = None
            if OUT_MODE == "xbar":
                oT = ot_pool.tile([P, sq], bf16, tag="oT")

            # attention per q-chunk, software-pipelined
            for c in range(NC_Q):
                qsl = slice(c * QCHUNK, (c + 1) * QCHUNK)
                a_chunk = a_pool.tile([P, NT_KV, QCHUNK], bf16, tag="a")
                h_ps = h_psum.tile([P, QCHUNK], f32, tag="h")
                e_pair = None
                for t in range(NT_KV + LA):
                    if t < NT_KV:
                        if t % 2 == 0:
                            e_pair = e_psum.tile([P, 2, QCHUNK], f32, tag="e")
                        nc.tensor.matmul(e_pair[:, t % 2, :],
                                         lhsT=xkvT[:, t * P:(t + 1) * P],
                                         rhs=pt[:, qsl],
                                         start=True, stop=True)
                        if t % 2 == 1:
                            nc.scalar.activation(
                                a_chunk[:, t - 1:t + 1, :].rearrange(
                                    "p a b -> p (a b)"),
                                e_pair[:].rearrange("p a b -> p (a b)"),
                                mybir.ActivationFunctionType.Exp, scale=SCALE)
                    if t == 6:
                        flush()
                    if t >= LA and t - LA < NT_KV:
                        tt = t - LA
                        nc.tensor.matmul(h_ps[:], lhsT=xkv_sb[:, tt, :],
                                         rhs=a_chunk[:, tt, :],
                                         start=(tt == 0), stop=(tt == NT_KV - 1))
                st = {"a_chunk": a_chunk, "h_ps": h_ps, "b": b, "c": c,
                      "oT": oT}
                emit_schain(st)
                pending.append(st)

        flush()

    nc.compile()
    return nc


def get_program(n_batch=B_PER_CORE, sq=SQ, skv=SKV, n_iters=1):
    key = (n_batch, sq, skv, S_MODE, OUT_MODE, n_iters)
    if key not in _PROGRAM_CACHE:
        _PROGRAM_CACHE[key] = build_program(n_batch, sq, skv, n_iters)
    return _PROGRAM_CACHE[key]


def _host_consts(Wv, bv, Wk, bk, Wq, bq, Wd, bd):
    import ml_dtypes
    f64 = np.float64
    Wq64, Wk64 = np.asarray(Wq, f64), np.asarray(Wk, f64)
    Wv64, Wd64 = np.asarray(Wv, f64), np.asarray(Wd, f64)
    wqk = (Wq64.T @ Wk64)                      # [din_q, din_k] -> PT lhsT
    mdv = (Wd64 @ Wv64).T                      # [d, j]: oproj stationary
    ccol = (Wk64.T @ np.asarray(bq, f64)).reshape(D, 1)  # PT per-part. bias
    btil = Wd64 @ np.asarray(bv, f64) + np.asarray(bd, f64)
    return {
        "wqk": np.ascontiguousarray(wqk.astype(ml_dtypes.bfloat16)),
        "mdv": np.ascontiguousarray(mdv.astype(ml_dtypes.bfloat16)),
        "ccol": np.ascontiguousarray(ccol.astype(np.float32)),
        "btcol": np.ascontiguousarray(btil.reshape(D, 1).astype(np.float32)),
        "bb4": np.ascontiguousarray(np.tile(btil, 4).reshape(1, 4 * D)
                                    .astype(np.float32)),
        "onescol": np.ones((D, 1), np.float32),
    }


def kernel(smiles_features, image_features, Wv, bv, Wk, bk, Wq, bq, Wd, bd,
           _trace=False):
    import ml_dtypes
    from concourse.bass_utils import run_bass_kernel_spmd

    xkv = np.ascontiguousarray(
        np.asarray(smiles_features, np.float32).astype(ml_dtypes.bfloat16))
    xq = np.ascontiguousarray(
        np.asarray(image_features, np.float32).astype(ml_dtypes.bfloat16))
    consts = _host_consts(Wv, bv, Wk, bk, Wq, bq, Wd, bd)

    nc = get_program()
    in_maps = []
    for core in range(N_CORES):
        lo = core * B_PER_CORE
        hi = lo + B_PER_CORE
        m = dict(consts)
        m["xq"] = xq[lo:hi]
        m["xkv"] = xkv[lo:hi]
        in_maps.append(m)

    res = run_bass_kernel_spmd(nc, in_maps, list(range(N_CORES)),
                               trace=_trace)
    out = np.concatenate([r["out"] for r in res.results], axis=0)
    if _trace:
        return out, res
    return out


# revision 11
# speedup vs baseline: 1.2846x; 1.2846x over previous
"""Trainium2 Bass kernel for CrossAttentionFusion.

Reference computation (per batch element b, torch Linear convention):
    V = Xkv @ Wv.T + bv            [Skv, D]
    K = Xkv @ Wk.T + bk            [Skv, D]
    Q = Xq  @ Wq.T + bq            [Sq, D]
    E = Q @ K.T / sqrt(128)        [Sq, Skv]
    A = softmax(E, axis=-1)
    F = A @ V                      [Sq, D]
    O = F @ Wd.T + bd              [Sq, D]

Sharding: data-parallel over batch, B=32 across 8 cores (4 per core).

This environment executes NEFFs with a large per-instruction overhead
(~50-120us/instr, engines overlapping), so the design minimizes
instruction count per engine rather than modeled cycles:

  Algebraic folds (exact):
    E = Q K^T = (Xq Wqk + 1 c^T) Xkv^T + r 1^T,  Wqk = Wq^T Wk, c = Wk^T bq
      (the r 1^T term is constant per-row -> cancels in softmax, dropped)
    O^T = M^T H^T / S + btil 1^T,  H = A_unnorm Xkv,  M = (Wd Wv)^T,
          btil = Wd bv + bd
  so the K and V projections disappear and the O projection is ONE
  stationary-M matmul per q-chunk in the transposed domain.
  Wqk/M/c/btil are precomputed on host (128x128, negligible).

  Inputs are cast to bf16 on host (error ~0.1%, budget 2e-2):
    - X^T obtained with ONE XBAR dma-transpose per tensor (no PE transposes)
    - O^T transposed back by ONE XBAR SBUF->SBUF dma-transpose per batch,
      then written out by a Pool (SWDGE) casting DMA bf16->f32
    - all big matmuls run bf16 at 1 cycle/row

  Softmax denominator: A-chunk tiles live contiguously [128,16,512];
  ONE strided DVE reduce sums the 16 kv tiles, Pool partition_all_reduce
  sums the 128 partitions (broadcasting the result), DVE reciprocal, and
  the normalization is folded into the single DVE instruction that moves
  H^T out of PSUM. exp skips max-subtraction (E ~ N(0,1), safe in fp32).

  Per batch (Sq=Skv=2048):
    PE: 4 PT + 64 E + 64 AV + 4 Oproj            = 136
    DVE: 4 PT-bias + 4 Sred + 4 recip + 4 Hmul + 4 Obias = 20
    ACT: 32 exp (1024-wide pairs)
    Pool: 4 all_reduce + 1 cast-DMA;  DMA: 4

  The PE tail (Oproj) of chunk c is deferred into chunk c+1's E-loop so
  the cross-engine S-chain latency never blocks the PE stream.
"""

import os
import numpy as np

B_TOTAL = 32
N_CORES = 8
B_PER_CORE = B_TOTAL // N_CORES
SQ = 2048
SKV = 2048
D = 128
P = 128
QCHUNK = 512
LA = 2  # AV trails E by LA kv-tiles
SCALE = 1.0 / np.sqrt(128.0)

# A/B switches (env):
#   BASS_S_MODE   = allred | reduce   (Pool all_reduce vs PE colsum chain)
#   BASS_OUT_MODE = xbar | direct     (O^T + xbar-out vs q-major oproj)
S_MODE = os.environ.get("BASS_S_MODE", "allred")
OUT_MODE = os.environ.get("BASS_OUT_MODE", "xbar")

_PROGRAM_CACHE = {}


def build_program(n_batch=B_PER_CORE, sq=SQ, skv=SKV, n_iters=1):
    import concourse.mybir as mybir
    import concourse.tile as tile
    from concourse import bacc, bass_isa
    from contextlib import ExitStack

    f32 = mybir.dt.float32
    bf16 = mybir.dt.bfloat16

    NT_Q = sq // P        # 16 q tiles per batch
    NT_KV = skv // P      # 16 kv tiles per batch
    NC_Q = sq // QCHUNK   # 4 q chunks per batch
    QSUB = QCHUNK // P    # 4 q subtiles per chunk

    nc = bacc.Bacc("TRN2", target_bir_lowering=False, debug=False)

    xq_d = nc.dram_tensor("xq", [n_batch, sq, D], bf16, kind="ExternalInput")
    xkv_d = nc.dram_tensor("xkv", [n_batch, skv, D], bf16, kind="ExternalInput")
    wqk_d = nc.dram_tensor("wqk", [D, D], bf16, kind="ExternalInput")
    m_d = nc.dram_tensor("mdv", [D, D], bf16, kind="ExternalInput")
    ccol_d = nc.dram_tensor("ccol", [D, 1], f32, kind="ExternalInput")
    btcol_d = nc.dram_tensor("btcol", [D, 1], f32, kind="ExternalInput")
    bb4_d = nc.dram_tensor("bb4", [1, QSUB * D], f32, kind="ExternalInput")
    ones_d = nc.dram_tensor("onescol", [D, 1], f32, kind="ExternalInput")
    out_d = nc.dram_tensor("out", [n_batch, sq, D], f32, kind="ExternalOutput")

    with tile.TileContext(nc) as tc, ExitStack() as ctx:
        const = ctx.enter_context(tc.tile_pool(name="const", bufs=1))
        xt_pool = ctx.enter_context(tc.tile_pool(name="xt", bufs=2))
        xin_pool = ctx.enter_context(tc.tile_pool(name="xin", bufs=2))
        pt_pool = ctx.enter_context(tc.tile_pool(name="pt", bufs=2))
        a_pool = ctx.enter_context(tc.tile_pool(name="a", bufs=2))
        sp_pool = ctx.enter_context(tc.tile_pool(name="sp", bufs=2))
        rb_pool = ctx.enter_context(tc.tile_pool(name="rb", bufs=2))
        ht_pool = ctx.enter_context(tc.tile_pool(name="ht", bufs=2))
        ot_pool = ctx.enter_context(tc.tile_pool(name="ot", bufs=2))
        os_pool = ctx.enter_context(tc.tile_pool(name="os", bufs=2))
        e_psum = ctx.enter_context(tc.tile_pool(name="e_psum", bufs=2, space="PSUM"))
        h_psum = ctx.enter_context(tc.tile_pool(name="h_psum", bufs=2, space="PSUM"))
        op_psum = ctx.enter_context(tc.tile_pool(name="op_psum", bufs=2, space="PSUM"))

        # ---- constants (host-precomputed, just DMA'd in) ----
        wqk_sb = const.tile([D, D], bf16, tag="wqk")
        nc.sync.dma_start(wqk_sb[:], wqk_d[:, :])
        m_sb = const.tile([D, D], bf16, tag="mdv")
        nc.sync.dma_start(m_sb[:], m_d[:, :])
        ccol = const.tile([D, 1], f32, tag="ccol")
        nc.sync.dma_start(ccol[:], ccol_d[:, :])
        btcol = const.tile([D, 1], f32, tag="btcol")
        nc.sync.dma_start(btcol[:], btcol_d[:, :])
        if OUT_MODE == "direct":
            bb4_row = const.tile([1, QSUB * D], f32, tag="bb4r")
            nc.sync.dma_start(bb4_row[:], bb4_d[:, :])
            bbc4 = const.tile([P, QSUB * D], f32, tag="bb4")
            nc.gpsimd.partition_broadcast(bbc4[:], bb4_row[:], channels=P)
        if S_MODE == "reduce":
            ones_col = const.tile([D, 1], f32, tag="ones")
            nc.sync.dma_start(ones_col[:], ones_d[:, :])
            s_psum = ctx.enter_context(
                tc.tile_pool(name="s_psum", bufs=1, space="PSUM"))
            r_pool = ctx.enter_context(tc.tile_pool(name="r", bufs=2))

        # deferred PE tail (O-projection) of the previous chunk
        pending = []

        def emit_schain(st):
            """S-chain + H normalization; no PE instructions (allred mode)."""
            a_chunk, h_ps = st["a_chunk"], st["h_ps"]
            SP = sp_pool.tile([P, QCHUNK], f32, tag="sp")
            nc.vector.tensor_reduce(
                SP[:], a_chunk[:].rearrange("p t q -> p q t"),
                mybir.AxisListType.X, mybir.AluOpType.add)
            rb = rb_pool.tile([P, QCHUNK], f32, tag="rb")
            if S_MODE == "allred":
                sb = sp_pool.tile([P, QCHUNK], f32, tag="sb")
                nc.gpsimd.partition_all_reduce(
                    sb[:], SP[:], channels=P, reduce_op=bass_isa.ReduceOp.add)
                nc.vector.reciprocal(rb[:], sb[:])
            else:
                s_ps = s_psum.tile([1, QCHUNK], f32, tag="s")
                nc.tensor.matmul(s_ps[0:1, :], lhsT=ones_col[:], rhs=SP[:],
                                 start=True, stop=True)
                recip = r_pool.tile([1, QCHUNK], f32, tag="r")
                nc.vector.reciprocal(recip[:], s_ps[0:1, :])
                nc.gpsimd.partition_broadcast(rb[:], recip[:], channels=P)
            ht = ht_pool.tile([P, QCHUNK], bf16, tag="ht")
            nc.vector.tensor_mul(ht[:], h_ps[:], rb[:])
            st["ht"] = ht

        def emit_oproj(st):
            b, c, ht = st["b"], st["c"], st["ht"]
            if OUT_MODE == "xbar":
                # O^T chunk = M^T H^T: ONE stationary-M matmul
                op_ps = op_psum.tile([P, QCHUNK], f32, tag="op")
                nc.tensor.matmul(op_ps[:], lhsT=m_sb[:], rhs=ht[:],
                                 start=True, stop=True)
                nc.vector.tensor_scalar_add(
                    st["oT"][:, c * QCHUNK:(c + 1) * QCHUNK], op_ps[:],
                    btcol[:])
            else:
                op_ps = op_psum.tile([P, QSUB, D], f32, tag="op")
                for j in range(QSUB):
                    nc.tensor.matmul(op_ps[:, j, :],
                                     lhsT=ht[:, j * P:(j + 1) * P],
                                     rhs=m_sb[:], start=True, stop=True)
                o_sb = os_pool.tile([P, QSUB, D], f32, tag="o")
                nc.vector.tensor_add(
                    o_sb[:].rearrange("p a b -> p (a b)"),
                    op_ps[:].rearrange("p a b -> p (a b)"),
                    bbc4[:])
                o_dst = out_d[b, c * QCHUNK:(c + 1) * QCHUNK, :].rearrange(
                    "(t p) d -> p t d", p=P)
                nc.sync.dma_start(o_dst, o_sb[:])
            if OUT_MODE == "xbar" and c == NC_Q - 1:
                # all 4 chunks of batch b written: transpose + cast out
                o_seq = os_pool.tile([P, NT_Q, D], bf16, tag="oseq")
                nc.sync.dma_start_transpose(o_seq[:], st["oT"][:])
                nc.gpsimd.dma_start(
                    out_d[b].rearrange("(t p) d -> p t d", p=P), o_seq[:])

        def flush():
            while pending:
                emit_oproj(pending.pop(0))

        # ---- per batch (n_iters>1 only for wall-clock HW timing) ----
        def issue_input_dmas(b):
            """X^T via XBAR dma-transpose; Xkv seq-major via plain DMA."""
            xkvT = xt_pool.tile([P, skv], bf16, tag="xkvT")
            xqT = xt_pool.tile([P, sq], bf16, tag="xqT")
            nc.sync.dma_start_transpose(xkvT[:], xkv_d[b])
            nc.sync.dma_start_transpose(xqT[:], xq_d[b])
            xkv_sb = xin_pool.tile([P, NT_KV, D], bf16, tag="xin")
            nc.sync.dma_start(xkv_sb[:], xkv_d[b].rearrange(
                "(t p) d -> p t d", p=P))
            return xkvT, xqT, xkv_sb

        batch_seq = [bb for _ in range(n_iters) for bb in range(n_batch)]
        prefetch = os.environ.get("BASS_PREFETCH", "0") == "1"
        prefetched = issue_input_dmas(batch_seq[0]) if prefetch else None
        for bi, b in enumerate(batch_seq):
            if prefetch:
                xkvT, xqT, xkv_sb = prefetched
                if bi + 1 < len(batch_seq):
                    # issue next batch's input DMAs now (bufs=2 pools); the
                    # DMA queue fills them while this batch computes
                    prefetched = issue_input_dmas(batch_seq[bi + 1])
            else:
                xkvT, xqT, xkv_sb = issue_input_dmas(b)

            # PT = Wqk^T Xq^T + c  [D, Sq] (feature-major q projection)
            pt = pt_pool.tile([P, sq], bf16, tag="pt")
            for cq in range(sq // QCHUNK):
                ps = h_psum.tile([P, QCHUNK], f32, tag="h")
                nc.tensor.matmul(ps[:], lhsT=wqk_sb[:],
                                 rhs=xqT[:, cq * QCHUNK:(cq + 1) * QCHUNK],
                                 start=True, stop=True)
                nc.vector.tensor_scalar_add(
                    pt[:, cq * QCHUNK:(cq + 1) * QCHUNK], ps[:], ccol[:])

            oT = None
            if OUT_MODE == "xbar":
                oT = ot_pool.tile([P, sq], bf16, tag="oT")

            # attention per q-chunk, software-pipelined
            for c in range(NC_Q):
                qsl = slice(c * QCHUNK, (c + 1) * QCHUNK)
                a_chunk = a_pool.tile([P, NT_KV, QCHUNK], bf16, tag="a")
                h_ps = h_psum.tile([P, QCHUNK], f32, tag="h")
                e_pair = None
                for t in range(NT_KV + LA):
                    if t < NT_KV:
                        if t % 2 == 0:
                            e_pair = e_psum.tile([P, 2, QCHUNK], f32, tag="e")
                        nc.tensor.matmul(e_pair[:, t % 2, :],
                                         lhsT=xkvT[:, t * P:(t + 1) * P],
                                         rhs=pt[:, qsl],
                                         start=True, stop=True)
                        if t % 2 == 1:
                            nc.scalar.activation(
                                a_chunk[:, t - 1:t + 1, :].rearrange(
                                    "p a b -> p (a b)"),
                                e_pair[:].rearrange("p a b -> p (a b)"),
                                mybir.ActivationFunctionType.Exp, scale=SCALE)
                    if t == 6:
                        flush()
                    if t >= LA and t - LA < NT_KV:
                        tt = t - LA
                        nc.tensor.matmul(h_ps[:], lhsT=xkv_sb[:, tt, :],
                                         rhs=a_chunk[:, tt, :],
                                         start=(tt == 0), stop=(tt == NT_KV - 1))
                st = {"a_chunk": a_chunk, "h_ps": h_ps, "b": b, "c": c,
                      "oT": oT}
                emit_schain(st)
                pending.append(st)

        flush()

    nc.compile()
    return nc


def get_program(n_batch=B_PER_CORE, sq=SQ, skv=SKV, n_iters=1):
    key = (n_batch, sq, skv, S_MODE, OUT_MODE,
           os.environ.get("BASS_PREFETCH", "0"), n_iters)
    if key not in _PROGRAM_CACHE:
        _PROGRAM_CACHE[key] = build_program(n_batch, sq, skv, n_iters)
    return _PROGRAM_CACHE[key]


def _host_consts(Wv, bv, Wk, bk, Wq, bq, Wd, bd):
    import ml_dtypes
    f64 = np.float64
    Wq64, Wk64 = np.asarray(Wq, f64), np.asarray(Wk, f64)
    Wv64, Wd64 = np.asarray(Wv, f64), np.asarray(Wd, f64)
    wqk = (Wq64.T @ Wk64)                      # [din_q, din_k] -> PT lhsT
    mdv = (Wd64 @ Wv64).T                      # [d, j]: oproj stationary
    ccol = (Wk64.T @ np.asarray(bq, f64)).reshape(D, 1)  # PT per-part. bias
    btil = Wd64 @ np.asarray(bv, f64) + np.asarray(bd, f64)
    return {
        "wqk": np.ascontiguousarray(wqk.astype(ml_dtypes.bfloat16)),
        "mdv": np.ascontiguousarray(mdv.astype(ml_dtypes.bfloat16)),
        "ccol": np.ascontiguousarray(ccol.astype(np.float32)),
        "btcol": np.ascontiguousarray(btil.reshape(D, 1).astype(np.float32)),
        "bb4": np.ascontiguousarray(np.tile(btil, 4).reshape(1, 4 * D)
                                    .astype(np.float32)),
        "onescol": np.ones((D, 1), np.float32),
    }


def kernel(smiles_features, image_features, Wv, bv, Wk, bk, Wq, bq, Wd, bd,
           _trace=False):
    import ml_dtypes
    from concourse.bass_utils import run_bass_kernel_spmd

    xkv = np.ascontiguousarray(
        np.asarray(smiles_features, np.float32).astype(ml_dtypes.bfloat16))
    xq = np.ascontiguousarray(
        np.asarray(image_features, np.float32).astype(ml_dtypes.bfloat16))
    consts = _host_consts(Wv, bv, Wk, bk, Wq, bq, Wd, bd)

    nc = get_program()
    in_maps = []
    for core in range(N_CORES):
        lo = core * B_PER_CORE
        hi = lo + B_PER_CORE
        m = dict(consts)
        m["xq"] = xq[lo:hi]
        m["xkv"] = xkv[lo:hi]
        in_maps.append(m)

    res = run_bass_kernel_spmd(nc, in_maps, list(range(N_CORES)),
                               trace=_trace)
    out = np.concatenate([r["out"] for r in res.results], axis=0)
    if _trace:
        return out, res
    return out
